# revision 84
# baseline (speedup 1.0000x reference)
# DETR multi-head dot-product attention for Trainium2 (Bass/Tile), 8 NeuronCores.
#
# Problem (hardcoded): B=4, S=1024, D=1024, H=16, HD=64, f32.
#   q = (inputs_q + pos_emb_q) @ wq + bq;  q /= sqrt(HD)
#   k = (inputs_kv + pos_emb_k) @ wk + bk
#   v = (inputs_kv + pos_emb_v) @ wv + bv          (bv == 0 by problem spec)
#   attn = softmax(q k^T + key_padding_bias); out = (attn v) @ wo + bo
#
# Sharding: 8 cores = 4 batches x 2 head-groups of 8 heads. Each core computes
# its batch's projections restricted to its head-group's features (512 of 1024),
# full attention for its 8 heads, and a partial output projection. The host
# sums the two head-group partials per batch (partials ship bf16).
#
# Speed structure (vs the 117.9us bf16 version):
#  - Q/K/V projections run as fp8e4m3 DoubleRow matmuls with hi+lo error
#    compensation: x ~= x_hi + x_lo (both e4m3), w scaled by 256 then split
#    the same way; x@w ~= (xh@wh + xh@wl + xl@wh) * 2^-8 with the 2^-8 and
#    the bias folded into the DVE fixup. DoubleRow contracts two 128-chunks
#    per instruction at 0.5 cycles/row: 25% less PE time per projection at
#    bf16-equivalent accuracy. hi/lo ship interleaved in one tensor so each
#    DMA piece feeds complete matmuls.
#  - The attention path stays f32r (QK) / bf16 (pt, AV, O): fp8 anywhere
#    there blows the 2e-2 error budget (softmax here is sharp).
#  - Input DMAs split across two queues (weights on SP, activations on ACT,
#    which is idle until the first exp) so descriptor generation
#    parallelizes; outputs ride SP/Pool alternately and ship bf16.
#  - First K chain runs pair-major so PE start is paced by the first 512KB.
#  - All O-projection work that does not depend on the last heads' attention
#    is pulled ahead of the last QK pairs, so the post-exp drain is only:
#    fine-grained AV of the last two heads + the final head-pair matmuls.

import sys

for _p in ("/opt/trn_rl_repo", "/root/.axon_site/_ro/trn_rl_repo"):
    if _p not in sys.path:
        sys.path.append(_p)

import numpy as np
import ml_dtypes

import concourse.bass as bass
import concourse.mybir as mybir
import concourse.tile as tile
from concourse import bacc
from concourse.bass_utils import run_bass_kernel_spmd

B, S, D = 4, 1024, 1024
H, HD = 16, 64
F = 512          # features per head-group core (8 heads * 64)
NH = 8           # heads per core
P = 128          # partitions
KC = D // P      # contraction chunks for the input projections (8)
SC = S // P      # key chunks (8)
SH = 512         # S-half (query block per attention slot)

f32 = mybir.dt.float32
b16 = mybir.dt.bfloat16
f8 = mybir.dt.float8e4
Exp = mybir.ActivationFunctionType.Exp
MUL = mybir.AluOpType.mult
ADD = mybir.AluOpType.add
DR = mybir.MatmulPerfMode.DoubleRow
FIX = 1.0 / 256.0   # post-scale compensating the x256 fp8 weight prescale


def build_program(repeat=1, debug_taps=False):
    nc = bacc.Bacc("TRN2", target_bir_lowering=False, debug=False)
    f32r = mybir.dt.float32r
    dbg = {}
    if debug_taps:
        dbg["kt"] = nc.dram_tensor("dbg_kt", [P, 4, S], f32r, kind="ExternalOutput")
        dbg["qt"] = nc.dram_tensor("dbg_qt", [P, 4, S], f32r, kind="ExternalOutput")
        dbg["vsb"] = nc.dram_tensor("dbg_vsb", [P, SC, NH, HD + 1], b16,
                                    kind="ExternalOutput")
        dbg["pt0"] = nc.dram_tensor("dbg_pt0", [P, SC, SH], b16,
                                    kind="ExternalOutput")
        dbg["xt"] = nc.dram_tensor("dbg_xt", [P, 4, S], b16,
                                   kind="ExternalOutput")

    # fp8 activations (transposed [feature, {hi,lo}, s]) and weights
    qin_d = nc.dram_tensor("qin8", [D, 2, S], f8, kind="ExternalInput")
    kin_d = nc.dram_tensor("kin8", [D, 2, S], f8, kind="ExternalInput")
    vin_d = nc.dram_tensor("vin8", [D, 2, S], f8, kind="ExternalInput")
    wq_d = nc.dram_tensor("wq8", [D, 2, F], f8, kind="ExternalInput")
    wk_d = nc.dram_tensor("wk8", [D, 2, F], f8, kind="ExternalInput")
    wv_d = nc.dram_tensor("wv8", [D, 2, F], f8, kind="ExternalInput")
    wo_d = nc.dram_tensor("wo", [F, D], b16, kind="ExternalInput")
    bq_d = nc.dram_tensor("bq", [F], f32, kind="ExternalInput")  # x32 on host
    bk_d = nc.dram_tensor("bk", [F], f32, kind="ExternalInput")  # x256 on host
    bo_d = nc.dram_tensor("bo", [D], f32, kind="ExternalInput")
    mk_d = nc.dram_tensor("mk", [S], f32, kind="ExternalInput")  # mask x 2^-8
    # mask replicated per head for V's extra (denominator) column, unscaled
    vones_d = nc.dram_tensor("vones", [P, SC, NH], b16, kind="ExternalInput")
    ident_d = nc.dram_tensor("ident", [P, P], b16, kind="ExternalInput")
    out_d = nc.dram_tensor("out_t", [D, S], b16, kind="ExternalOutput")

    with tile.TileContext(nc) as tc:
        with (
            tc.tile_pool(name="persist", bufs=1) as persist,
            tc.tile_pool(name="wmat", bufs=1) as w_pool,
            tc.tile_pool(name="acts", bufs=5) as acts_pool,
            tc.tile_pool(name="ptp", bufs=7) as pt_pool,
            tc.tile_pool(name="xnp", bufs=3) as xn_pool,
            tc.tile_pool(name="rcpp", bufs=3) as rcp_pool,
            tc.tile_pool(name="outb", bufs=8) as ob_pool,
            tc.tile_pool(name="pslg", bufs=2, space=bass.MemorySpace.PSUM) as pslg,
            tc.tile_pool(name="flex", bufs=4, space=bass.MemorySpace.PSUM) as flex,
        ):
            # ---- persistent tiles ----
            # f32r: same PE cost as bf16 at free>=256, avoids an extra bf16
            # rounding of the projected Q/K before the logits matmuls.
            qt = persist.tile([P, 4, S], f32r, tag="qt")    # Q^T  [feature, s]
            kt = persist.tile([P, 4, S], f32r, tag="kt")    # K^T  [feature, s]
            xt = persist.tile([P, 4, S], b16, tag="xt")     # attn-out^T, normed
            # V in natural layout [s, head, hd] with a mask column per head.
            vsb = persist.tile([P, SC, NH, HD + 1], b16, tag="vsb")
            po0 = persist.tile([P, KC, SH], b16, tag="po0")  # O-sh0 hp01 part
            po1 = persist.tile([P, KC, SH], b16, tag="po1")  # O-sh1 hp01 part
            bq_sb = persist.tile([P, 4], f32, tag="bq")
            bk_sb = persist.tile([P, 4], f32, tag="bk")
            bo_sb = persist.tile([P, KC], f32, tag="bo")
            mk_sb = persist.tile([P, SC], f32, tag="mk")
            id_sb = persist.tile([P, P], b16, tag="ident")
            vo_sb = persist.tile([P, SC, NH], b16, tag="vones")

            for _rep in range(repeat):
                # ================= DMA streams =================
                # Three DMA queues carry the input stream in parallel:
                # weights on SP (HWDGE), K-activations on ACT (HWDGE,
                # idle until the first exp), Q/V-activations on the
                # gpsimd SWDGE queue. DMA APs allow at most 3 dims, so hi/lo
                # pieces load separately into [P, {hi,lo}, KC, *] tiles.
                def load_act(eng, dst, src_d, sh, t, c0, c1):
                    eng.dma_start(
                        dst[:, t, c0:c1, :],
                        src_d[c0 * P:c1 * P, t,
                              sh * SH:(sh + 1) * SH].rearrange(
                                  "(c p) s -> p c s", p=P))

                def load_wt(eng, dst, src_d, t, c0, c1):
                    eng.dma_start(
                        dst[:, t, c0:c1, :],
                        src_d[c0 * P:c1 * P, t, :].rearrange(
                            "(c p) f -> p c f", p=P))

                # weights tiles ([partition, {hi,lo}, chunk, cols])
                wk2 = w_pool.tile([P, 2, KC, F], f8, tag="wk2")
                wq2 = w_pool.tile([P, 2, KC, F], f8, tag="wq2")
                wv2 = w_pool.tile([P, 2, KC, F], f8, tag="wv2")
                wo_sb = w_pool.tile([P, 4, D], b16, tag="wo")

                def act(name):
                    return acts_pool.tile([P, 2, KC, SH], f8, tag="acts",
                                          name=name)

                k0, k1 = act("k0"), act("k1")
                q0, q1 = act("q0"), act("q1")
                v0, v1 = act("v0"), act("v1")
                # DMA transfers are globally serial, so order the stream by
                # first use: hi pieces of wk+k0 first (the term-major K0
                # chain runs all hi*hi products before touching lo), then
                # the lo pieces, then k1, wq+q0, wv+v0, v1, q1, wo.
                for half in (0, 1):
                    c0, c1 = half * 4, half * 4 + 4
                    load_wt(nc.sync, wk2, wk_d, 0, c0, c1)
                    load_act(nc.scalar, k0, kin_d, 0, 0, c0, c1)
                for half in (0, 1):
                    c0, c1 = half * 4, half * 4 + 4
                    load_act(nc.scalar, k0, kin_d, 0, 1, c0, c1)
                    load_wt(nc.sync, wk2, wk_d, 1, c0, c1)
                nc.sync.dma_start(bk_sb[:], bk_d[:].rearrange("(m p) -> p m", p=P))
                nc.sync.dma_start(bq_sb[:], bq_d[:].rearrange("(m p) -> p m", p=P))
                for t in range(2):
                    load_act(nc.scalar, k1, kin_d, 1, t, 0, KC)
                nc.sync.dma_start(mk_sb[:], mk_d[:].rearrange("(c p) -> p c", p=P))
                # vones loads contiguously, then a DVE copy scatters it into
                # vsb's stride-65 mask column (a direct strided DMA costs
                # 3.6us of serial descriptor time)
                nc.sync.dma_start(vo_sb[:], vones_d[:])
                nc.vector.tensor_copy(vsb[:, :, :, HD], vo_sb[:])
                for t in range(2):
                    load_wt(nc.sync, wq2, wq_d, t, 0, KC)
                    load_act(nc.scalar, q0, qin_d, 0, t, 0, KC)
                for t in range(2):
                    load_wt(nc.sync, wv2, wv_d, t, 0, KC)
                    load_act(nc.scalar, v0, vin_d, 0, t, 0, KC)
                for t in range(2):
                    load_act(nc.scalar, v1, vin_d, 1, t, 0, KC)
                nc.sync.dma_start(bo_sb[:], bo_d[:].rearrange("(m p) -> p m", p=P))
                nc.sync.dma_start(id_sb[:], ident_d[:])
                for t in range(2):
                    load_act(nc.scalar, q1, qin_d, 1, t, 0, KC)
                nc.sync.dma_start(
                    wo_sb[:], wo_d[:].rearrange("(k p) f -> p k f", p=P))

                # ================= compute emitters =================
                TERMS = ((0, 0), (0, 1), (1, 0))  # (lhs hi/lo, rhs hi/lo)

                def emit_chain8(lhs, rhs, lcol, post):
                    # acc = sum over 4 chunk-pairs of the 3 fp8 cross terms
                    acc = flex.tile([P, SH], f32, tag="flex", name="acc")
                    i = 0
                    for p4 in range(4):
                        a, b = 2 * p4, 2 * p4 + 2
                        for li, ri in TERMS:
                            nc.tensor.matmul(
                                acc[:], lhs[:, li, a:b, lcol],
                                rhs[:, ri, a:b, :],
                                start=(i == 0), stop=(i == 11), perf_mode=DR)
                            i += 1
                    post(acc)

                def fix_kq(dst, sh, m, bias_sb):
                    def post(acc):
                        nc.vector.tensor_scalar(
                            dst[:, m, sh * SH:(sh + 1) * SH], acc[:],
                            bias_sb[:, m:m + 1], FIX, op0=ADD, op1=MUL)
                    return post

                def fix_v(sc):
                    def post(acc):
                        # mk pre-scaled by 2^-8 on the host
                        nc.vector.tensor_scalar(
                            vsb[:, sc, :, 0:HD],
                            acc[:].rearrange("p (h d) -> p h d", d=HD),
                            mk_sb[:, sc:sc + 1], None, op0=MUL)
                    return post

                def emit_kqchain(w2, x2, bias_sb, dst, sh, m):
                    emit_chain8(w2, x2, slice(m * P, (m + 1) * P),
                                fix_kq(dst, sh, m, bias_sb))

                def emit_vchain(x2, sh, s):
                    emit_chain8(x2, wv2, slice(s * P, (s + 1) * P),
                                fix_v(sh * 4 + s))

                def emit_k_termmajor(w2, src, bias_sb, dst, sh):
                    # all 4 m-chains term-major then pair-major: the hi*hi
                    # sweep only needs the hi pieces of the weights and
                    # activations (the first half of their DMA), so PE
                    # starts earlier
                    accs = [flex.tile([P, SH], f32, tag="flex", name="acc")
                            for _ in range(4)]
                    for ti, (li, ri) in enumerate(TERMS):
                        for p4 in range(4):
                            a, b = 2 * p4, 2 * p4 + 2
                            for m in range(4):
                                col = slice(m * P, (m + 1) * P)
                                nc.tensor.matmul(
                                    accs[m][:], w2[:, li, a:b, col],
                                    src[:, ri, a:b, :],
                                    start=(ti == 0 and p4 == 0),
                                    stop=(ti == 2 and p4 == 3), perf_mode=DR)
                    for m in range(4):
                        fix_kq(dst, sh, m, bias_sb)(accs[m])

                def emit_qk_pair(sh, h, ptt, cp):
                    # logits^T chunk-pair cp + exp into ptt
                    po = (h % 2) * HD
                    mq = h // 2
                    lg = pslg.tile([P, 2, SH], f32, tag="lg", name="lg")
                    for i in range(2):
                        c = 2 * cp + i
                        nc.tensor.matmul(
                            lg[:, i, :],
                            kt[po:po + HD, mq, c * P:(c + 1) * P],
                            qt[po:po + HD, mq, sh * SH:(sh + 1) * SH],
                            start=True, stop=True)
                    nc.scalar.activation(ptt[:, 2 * cp:2 * cp + 2, :], lg[:], Exp)

                def emit_av(sh, h, ptt):
                    # flipped AV: out[q, hd|denom]; denom comes from the mask
                    # column of V. Normalize per-partition into xn.
                    av = flex.tile([P, 4, HD + 1], f32, tag="flex", name="av")
                    for q4 in range(4):
                        for c in range(SC):
                            nc.tensor.matmul(
                                av[:, q4, :],
                                ptt[:, c, q4 * P:(q4 + 1) * P],
                                vsb[:, c, h, :],
                                start=(c == 0), stop=(c == SC - 1))
                    rcpt = rcp_pool.tile([P, 4], f32, tag="rcp", name="rcpt")
                    nc.vector.reciprocal(rcpt[:], av[:, :, HD])
                    xnt = xn_pool.tile([P, 4, HD], b16, tag="xn", name="xnt")
                    nc.vector.tensor_mul(
                        xnt[:], av[:, :, 0:HD],
                        rcpt[:].unsqueeze(2).broadcast_to([P, 4, HD]))
                    return xnt

                def emit_tr(sh, h, xnt):
                    # transpose xn [q, hd] -> xt [hd, q] (feature-major)
                    po = (h % 2) * HD
                    mq = h // 2
                    xtp = flex.tile([HD, SH], b16, tag="flex", name="xtp")
                    for q4 in range(4):
                        nc.tensor.matmul(
                            xtp[:, q4 * P:(q4 + 1) * P], xnt[:, q4, :], id_sb[:],
                            start=True, stop=True, is_transpose=True)
                    nc.vector.tensor_copy(
                        xt[po:po + HD, mq, sh * SH:(sh + 1) * SH], xtp[:])

                Copy = mybir.ActivationFunctionType.Copy

                def emit_o_stage1(sh, m):
                    # head-pairs 0,1 -> bf16 partial (bias folded in so the
                    # drain merge needs no scalar operand)
                    dst = po0 if sh == 0 else po1
                    acc = flex.tile([P, SH], f32, tag="flex", name="acc")
                    for hp in (0, 1):
                        nc.tensor.matmul(
                            acc[:], wo_sb[:, hp, m * P:(m + 1) * P],
                            xt[:, hp, sh * SH:(sh + 1) * SH],
                            start=(hp == 0), stop=(hp == 1))
                    nc.vector.tensor_scalar_add(dst[:, m, :], acc[:],
                                                bo_sb[:, m:m + 1])

                def emit_o_stage2(sh, m, on_act=False):
                    # head-pairs 2,3 + partial merge -> bf16 -> DMA
                    src = po0 if sh == 0 else po1
                    acc = flex.tile([P, SH], f32, tag="flex", name="acc")
                    nc.tensor.matmul(
                        acc[:], wo_sb[:, 2, m * P:(m + 1) * P],
                        xt[:, 2, sh * SH:(sh + 1) * SH],
                        start=True, stop=False)
                    nc.tensor.matmul(
                        acc[:], wo_sb[:, 3, m * P:(m + 1) * P],
                        xt[:, 3, sh * SH:(sh + 1) * SH],
                        start=False, stop=not on_act)
                    ob = ob_pool.tile([P, SH], b16, tag="ob", name="ob")
                    if on_act:
                        # sh1 drain: add the partial on PE (identity matmul)
                        # and cast on the now-idle ACT engine, halving the
                        # serial DVE merge stream at the tail
                        nc.tensor.matmul(acc[:], id_sb[:], src[:, m, :],
                                         start=False, stop=True)
                        nc.scalar.activation(ob[:], acc[:], Copy)
                    else:
                        # partial already carries the bias
                        nc.vector.tensor_add(ob[:], acc[:], src[:, m, :])
                    # alternate output queues so neither serializes the tail
                    eng = nc.gpsimd if m % 2 == 0 else nc.sync
                    eng.dma_start(
                        out_d[m * P:(m + 1) * P, sh * SH:(sh + 1) * SH], ob[:])

                # ================= attention stream tables =================
                slots = [(s // NH, s % NH) for s in range(16)]
                av_sched = {6: [0, 1], 7: [2, 3], 8: [4, 5], 9: [6, 7],
                            10: [8], 11: [9, 10], 12: [11], 13: [12],
                            14: [13]}
                tr_sched = {7: [0, 1], 8: [2, 3], 9: [4, 5], 10: [6, 7],
                            11: [8], 12: [9, 10], 13: [11], 14: [12],
                            15: [13]}
                # all O work that does not need the last two heads is packed
                # before slot 15's QK pairs
                chain_sched = {
                    2: [("v", 0, 0), ("v", 0, 1)],
                    3: [("v", 0, 2), ("v", 0, 3)],
                    4: [("v", 1, 0), ("v", 1, 1)],
                    5: [("v", 1, 2), ("v", 1, 3)],
                    6: [("q1", 0)], 7: [("q1", 1)], 8: [("q1", 2)],
                    9: [("q1", 3), ("o0a", 0)],
                    10: [("o0a", 1), ("o0a", 2), ("o0a", 3)],
                    11: [("o0a", 4), ("o0a", 5), ("o0b", 0)],
                    12: [("o0a", 6), ("o0a", 7), ("o0b", 1)],
                    13: [("o0b", 2), ("o0b", 3), ("o1a", 0)],
                    14: [("o0b", 4), ("o0b", 5), ("o1a", 1), ("o1a", 2)],
                    15: [("o0b", 6)],
                }

                def emit_chain(spec):
                    kind = spec[0]
                    if kind == "v":
                        emit_vchain(v0 if spec[1] == 0 else v1,
                                    spec[1], spec[2])
                    elif kind == "q1":
                        emit_kqchain(wq2, q1, bq_sb, qt, 1, spec[1])
                    elif kind == "o0a":
                        emit_o_stage1(0, spec[1])
                    elif kind == "o0b":
                        emit_o_stage2(0, spec[1])
                    elif kind == "o1a":
                        emit_o_stage1(1, spec[1])

                def emit_av_tr_fine(sh, h, ptt, fillers=()):
                    # last-slot variant: per-q-tile AV -> rcp -> norm -> tr ->
                    # copy pipeline, so the tail latency is one q-tile.
                    po = (h % 2) * HD
                    mq = h // 2
                    av = flex.tile([P, 4, HD + 1], f32, tag="flex", name="av")
                    xtp = flex.tile([HD, SH], b16, tag="flex", name="xtp")
                    fillers = list(fillers)

                    def avmm(q4):
                        for c in range(SC):
                            nc.tensor.matmul(
                                av[:, q4, :],
                                ptt[:, c, q4 * P:(q4 + 1) * P],
                                vsb[:, c, h, :],
                                start=(c == 0), stop=(c == SC - 1))

                    def avq(q4):
                        rcpt = rcp_pool.tile([P, 1], f32, tag="rcp", name="rcpt")
                        nc.vector.reciprocal(rcpt[:], av[:, q4, HD:HD + 1])
                        xnt = xn_pool.tile([P, HD], b16, tag="xn", name="xnt")
                        nc.vector.tensor_scalar(
                            xnt[:], av[:, q4, 0:HD], rcpt[:], None, op0=MUL)
                        return xnt

                    def trq(q4, xnt):
                        nc.tensor.matmul(
                            xtp[:, q4 * P:(q4 + 1) * P], xnt[:], id_sb[:],
                            start=True, stop=True, is_transpose=True)
                        cs = slice(sh * SH + q4 * P, sh * SH + (q4 + 1) * P)
                        nc.vector.tensor_copy(xt[po:po + HD, mq, cs],
                                              xtp[:, q4 * P:(q4 + 1) * P])

                    def fill():
                        if fillers:
                            emit_chain(fillers.pop(0))

                    avmm(0)
                    xs = [avq(0)]
                    avmm(1)
                    fill()
                    xs.append(avq(1))
                    trq(0, xs[0])
                    avmm(2)
                    fill()
                    xs.append(avq(2))
                    trq(1, xs[1])
                    avmm(3)
                    fill()
                    xs.append(avq(3))
                    trq(2, xs[2])
                    fill()
                    trq(3, xs[3])

                pts = {}
                xns = {}

                def new_pt(s):
                    pts[s] = pt_pool.tile([P, SC, SH], b16, tag="pt",
                                          name="ptt")
                    return pts[s]

                # ============ phase A: K, V0, Q0 + first two slots ============
                # K0 first (term-major, DMA-paced), then K1, then V half 0
                # (its data lands during K1), then Q0 with slot 0/1's QK
                # pairs interleaved so the exp stream starts as soon as
                # qt m0 exists.
                emit_k_termmajor(wk2, k0, bk_sb, kt, 0)
                for m in range(4):
                    emit_kqchain(wk2, k1, bk_sb, kt, 1, m)
                new_pt(0)
                new_pt(1)
                emit_kqchain(wq2, q0, bq_sb, qt, 0, 0)
                emit_qk_pair(0, 0, pts[0], 0)
                emit_qk_pair(0, 0, pts[0], 1)
                emit_kqchain(wq2, q0, bq_sb, qt, 0, 1)
                emit_qk_pair(0, 0, pts[0], 2)
                emit_qk_pair(0, 0, pts[0], 3)
                emit_kqchain(wq2, q0, bq_sb, qt, 0, 2)
                emit_qk_pair(0, 1, pts[1], 0)
                emit_qk_pair(0, 1, pts[1], 1)
                emit_kqchain(wq2, q0, bq_sb, qt, 0, 3)
                emit_qk_pair(0, 1, pts[1], 2)
                emit_qk_pair(0, 1, pts[1], 3)

                for s, (sh, h) in list(enumerate(slots))[2:]:
                    ptt = new_pt(s)
                    avs = [iter(av_sched.get(s, []))]
                    trs = [iter(tr_sched.get(s, []))]
                    chains = list(chain_sched.get(s, []))

                    def fill(n_chains):
                        for t in avs[0]:
                            psh, ph = slots[t]
                            xns[t] = emit_av(psh, ph, pts[t])
                            del pts[t]
                            break
                        for t in trs[0]:
                            psh, ph = slots[t]
                            emit_tr(psh, ph, xns.pop(t))
                            break
                        for _ in range(n_chains):
                            if chains:
                                emit_chain(chains.pop(0))

                    emit_qk_pair(sh, h, ptt, 0)
                    emit_qk_pair(sh, h, ptt, 1)
                    fill(1)
                    emit_qk_pair(sh, h, ptt, 2)
                    fill(1)
                    emit_qk_pair(sh, h, ptt, 3)
                    # drain any remaining scheduled work for this block
                    for t in avs[0]:
                        psh, ph = slots[t]
                        xns[t] = emit_av(psh, ph, pts[t])
                        del pts[t]
                    for t in trs[0]:
                        psh, ph = slots[t]
                        emit_tr(psh, ph, xns.pop(t))
                    while chains:
                        emit_chain(chains.pop(0))
                    if s == 15:
                        emit_av_tr_fine(1, 6, pts.pop(14),
                                        fillers=[("o1a", 3), ("o1a", 4),
                                                 ("o1a", 5)])
                    if debug_taps and s == 2:
                        nc.sync.dma_start(dbg["pt0"][:], pts[2][:])

                # ================= drain =================
                emit_av_tr_fine(1, 7, pts.pop(15),
                                fillers=[("o1a", 6), ("o1a", 7),
                                         ("o0b", 7)])
                for m in range(4):
                    emit_o_stage2(1, m)
                for m in range(4, KC):
                    emit_o_stage2(1, m, on_act=True)
                if debug_taps:
                    nc.sync.dma_start(dbg["kt"][:], kt[:])
                    nc.sync.dma_start(dbg["qt"][:], qt[:])
                    nc.sync.dma_start(dbg["vsb"][:], vsb[:])
                    nc.sync.dma_start(dbg["xt"][:], xt[:])

    nc.compile()
    return nc


_program = None
_last_in_maps = None


def _get_program():
    global _program
    if _program is None:
        _program = build_program()
    return _program


def _split_e4m3(x):
    """Interleaved hi/lo e4m3 split along a new axis 1: [d, 2, n]."""
    e4 = ml_dtypes.float8_e4m3
    x = np.ascontiguousarray(x)
    hi = x.astype(e4)
    lo = (x - hi.astype(np.float32)).astype(e4)
    return np.ascontiguousarray(np.stack([hi, lo], axis=1))


def kernel(inputs_q, inputs_kv, pos_emb_q, pos_emb_k, pos_emb_v,
           key_padding_mask, wq, bq, wk, bk, wv, bv, wo, bo):
    nc = _get_program()
    bf16 = ml_dtypes.bfloat16

    wqf = np.asarray(wq, np.float32).reshape(D, H * HD)
    wkf = np.asarray(wk, np.float32).reshape(D, H * HD)
    wvf = np.asarray(wv, np.float32).reshape(D, H * HD)
    wof = np.asarray(wo, np.float32).reshape(H * HD, D)
    bqf = np.asarray(bq, np.float32).reshape(H * HD)
    bkf = np.asarray(bk, np.float32).reshape(H * HD)
    bvf = np.asarray(bv, np.float32).reshape(H * HD)
    bof = np.asarray(bo, np.float32).reshape(D)
    # bv is structurally zero in this problem; it has no cheap slot in the
    # transposed dataflow, so refuse loudly rather than silently drop it.
    assert np.all(bvf == 0.0), "nonzero bv is not supported"

    scale = np.float32(1.0 / np.sqrt(HD))
    iq = np.asarray(inputs_q, np.float32)
    ikv = np.asarray(inputs_kv, np.float32)
    # positional embeddings folded on the host (f32, same math as reference)
    q_in = iq + np.asarray(pos_emb_q, np.float32)
    k_in = ikv + np.asarray(pos_emb_k, np.float32)
    v_in = ikv + np.asarray(pos_emb_v, np.float32)
    mask = np.asarray(key_padding_mask, np.float32)
    ident = np.eye(P, dtype=bf16)

    in_maps = []
    for b in range(B):
        q8 = _split_e4m3(q_in[b].T)
        k8 = _split_e4m3(k_in[b].T)
        v8 = _split_e4m3(v_in[b].T)
        mk = np.ascontiguousarray(mask[b])
        vones = np.ascontiguousarray(
            np.broadcast_to(mk.reshape(SC, P).T[:, :, None], (P, SC, NH))
        ).astype(bf16)
        for hg in range(2):
            sl = slice(hg * F, (hg + 1) * F)
            in_maps.append({
                "qin8": q8, "kin8": k8, "vin8": v8,
                "wq8": _split_e4m3(wqf[:, sl] * (256.0 * scale)),
                "wk8": _split_e4m3(wkf[:, sl] * 256.0),
                "wv8": _split_e4m3(wvf[:, sl] * 256.0),
                "wo": np.ascontiguousarray(wof[sl, :]).astype(bf16),
                "bq": np.ascontiguousarray(bqf[sl]) * (256.0 * scale),
                "bk": np.ascontiguousarray(bkf[sl]) * 256.0,
                "bo": bof if hg == 0 else np.zeros_like(bof),
                "mk": mk * np.float32(FIX),
                "vones": vones,
                "ident": ident,
            })

    global _last_in_maps
    _last_in_maps = in_maps
    res = run_bass_kernel_spmd(nc, in_maps, list(range(2 * B)))
    outs = [np.asarray(res.results[i]["out_t"], np.float32)
            for i in range(2 * B)]
    out = np.stack([(outs[2 * b] + outs[2 * b + 1]).T for b in range(B)])
    return np.ascontiguousarray(out, dtype=np.float32)


# revision 89
# speedup vs baseline: 1.0001x; 1.0001x over previous
# DETR multi-head dot-product attention for Trainium2 (Bass/Tile), 8 NeuronCores.
#
# Problem (hardcoded): B=4, S=1024, D=1024, H=16, HD=64, f32.
#   q = (inputs_q + pos_emb_q) @ wq + bq;  q /= sqrt(HD)
#   k = (inputs_kv + pos_emb_k) @ wk + bk
#   v = (inputs_kv + pos_emb_v) @ wv + bv          (bv == 0 by problem spec)
#   attn = softmax(q k^T + key_padding_bias); out = (attn v) @ wo + bo
#
# Sharding: 8 cores = 4 batches x 2 head-groups of 8 heads. Each core computes
# its batch's projections restricted to its head-group's features (512 of 1024),
# full attention for its 8 heads, and a partial output projection. The host
# sums the two head-group partials per batch (partials ship bf16).
#
# Speed structure (vs the 117.9us bf16 version):
#  - Q/K/V projections run as fp8e4m3 DoubleRow matmuls with hi+lo error
#    compensation: x ~= x_hi + x_lo (both e4m3), w scaled by 256 then split
#    the same way; x@w ~= (xh@wh + xh@wl + xl@wh) * 2^-8 with the 2^-8 and
#    the bias folded into the DVE fixup. DoubleRow contracts two 128-chunks
#    per instruction at 0.5 cycles/row: 25% less PE time per projection at
#    bf16-equivalent accuracy. hi/lo ship interleaved in one tensor so each
#    DMA piece feeds complete matmuls.
#  - The attention path stays f32r (QK) / bf16 (pt, AV, O): fp8 anywhere
#    there blows the 2e-2 error budget (softmax here is sharp).
#  - Input DMAs split across two queues (weights on SP, activations on ACT,
#    which is idle until the first exp) so descriptor generation
#    parallelizes; outputs ride SP/Pool alternately and ship bf16.
#  - First K chain runs pair-major so PE start is paced by the first 512KB.
#  - All O-projection work that does not depend on the last heads' attention
#    is pulled ahead of the last QK pairs, so the post-exp drain is only:
#    fine-grained AV of the last two heads + the final head-pair matmuls.

import sys

for _p in ("/opt/trn_rl_repo", "/root/.axon_site/_ro/trn_rl_repo"):
    if _p not in sys.path:
        sys.path.append(_p)

import numpy as np
import ml_dtypes

import concourse.bass as bass
import concourse.mybir as mybir
import concourse.tile as tile
from concourse import bacc
from concourse.bass_utils import run_bass_kernel_spmd

B, S, D = 4, 1024, 1024
H, HD = 16, 64
F = 512          # features per head-group core (8 heads * 64)
NH = 8           # heads per core
P = 128          # partitions
KC = D // P      # contraction chunks for the input projections (8)
SC = S // P      # key chunks (8)
SH = 512         # S-half (query block per attention slot)

f32 = mybir.dt.float32
b16 = mybir.dt.bfloat16
f8 = mybir.dt.float8e4
Exp = mybir.ActivationFunctionType.Exp
MUL = mybir.AluOpType.mult
ADD = mybir.AluOpType.add
DR = mybir.MatmulPerfMode.DoubleRow
FIX = 1.0 / 256.0   # post-scale compensating the x256 fp8 weight prescale


def build_program(repeat=1, debug_taps=False):
    nc = bacc.Bacc("TRN2", target_bir_lowering=False, debug=False)
    f32r = mybir.dt.float32r
    dbg = {}
    if debug_taps:
        dbg["kt"] = nc.dram_tensor("dbg_kt", [P, 4, S], f32r, kind="ExternalOutput")
        dbg["qt"] = nc.dram_tensor("dbg_qt", [P, 4, S], f32r, kind="ExternalOutput")
        dbg["vsb"] = nc.dram_tensor("dbg_vsb", [P, SC, NH, HD + 1], b16,
                                    kind="ExternalOutput")
        dbg["pt0"] = nc.dram_tensor("dbg_pt0", [P, SC, SH], b16,
                                    kind="ExternalOutput")
        dbg["xt"] = nc.dram_tensor("dbg_xt", [P, 4, S], b16,
                                   kind="ExternalOutput")

    # fp8 activations (transposed [feature, {hi,lo}, s]) and weights
    qin_d = nc.dram_tensor("qin8", [D, 2, S], f8, kind="ExternalInput")
    kin_d = nc.dram_tensor("kin8", [D, 2, S], f8, kind="ExternalInput")
    vin_d = nc.dram_tensor("vin8", [D, 2, S], f8, kind="ExternalInput")
    wq_d = nc.dram_tensor("wq8", [D, 2, F], f8, kind="ExternalInput")
    wk_d = nc.dram_tensor("wk8", [D, 2, F], f8, kind="ExternalInput")
    wv_d = nc.dram_tensor("wv8", [D, 2, F], f8, kind="ExternalInput")
    wo_d = nc.dram_tensor("wo", [F, D], b16, kind="ExternalInput")
    bq_d = nc.dram_tensor("bq", [F], f32, kind="ExternalInput")  # x32 on host
    bk_d = nc.dram_tensor("bk", [F], f32, kind="ExternalInput")  # x256 on host
    bo_d = nc.dram_tensor("bo", [D], f32, kind="ExternalInput")
    mk_d = nc.dram_tensor("mk", [S], f32, kind="ExternalInput")  # mask x 2^-8
    # mask replicated per head for V's extra (denominator) column, unscaled
    vones_d = nc.dram_tensor("vones", [P, SC, NH], b16, kind="ExternalInput")
    ident_d = nc.dram_tensor("ident", [P, P], b16, kind="ExternalInput")
    out_d = nc.dram_tensor("out_t", [D, S], b16, kind="ExternalOutput")

    with tile.TileContext(nc) as tc:
        with (
            tc.tile_pool(name="persist", bufs=1) as persist,
            tc.tile_pool(name="wmat", bufs=1) as w_pool,
            tc.tile_pool(name="acts", bufs=5) as acts_pool,
            tc.tile_pool(name="ptp", bufs=7) as pt_pool,
            tc.tile_pool(name="xnp", bufs=3) as xn_pool,
            tc.tile_pool(name="rcpp", bufs=3) as rcp_pool,
            tc.tile_pool(name="outb", bufs=8) as ob_pool,
            tc.tile_pool(name="pslg", bufs=2, space=bass.MemorySpace.PSUM) as pslg,
            tc.tile_pool(name="flex", bufs=4, space=bass.MemorySpace.PSUM) as flex,
        ):
            # ---- persistent tiles ----
            # f32r: same PE cost as bf16 at free>=256, avoids an extra bf16
            # rounding of the projected Q/K before the logits matmuls.
            qt = persist.tile([P, 4, S], f32r, tag="qt")    # Q^T  [feature, s]
            kt = persist.tile([P, 4, S], f32r, tag="kt")    # K^T  [feature, s]
            xt = persist.tile([P, 4, S], b16, tag="xt")     # attn-out^T, normed
            # V in natural layout [s, head, hd] with a mask column per head.
            vsb = persist.tile([P, SC, NH, HD + 1], b16, tag="vsb")
            po0 = persist.tile([P, KC, SH], b16, tag="po0")  # O-sh0 hp01 part
            po1 = persist.tile([P, KC, SH], b16, tag="po1")  # O-sh1 hp01 part
            bq_sb = persist.tile([P, 4], f32, tag="bq")
            bk_sb = persist.tile([P, 4], f32, tag="bk")
            bo_sb = persist.tile([P, KC], f32, tag="bo")
            mk_sb = persist.tile([P, SC], f32, tag="mk")
            id_sb = persist.tile([P, P], b16, tag="ident")
            vo_sb = persist.tile([P, SC, NH], b16, tag="vones")

            for _rep in range(repeat):
                # ================= DMA streams =================
                # Three DMA queues carry the input stream in parallel:
                # weights on SP (HWDGE), K-activations on ACT (HWDGE,
                # idle until the first exp), Q/V-activations on the
                # gpsimd SWDGE queue. DMA APs allow at most 3 dims, so hi/lo
                # pieces load separately into [P, {hi,lo}, KC, *] tiles.
                def load_act(eng, dst, src_d, sh, t, c0, c1):
                    eng.dma_start(
                        dst[:, t, c0:c1, :],
                        src_d[c0 * P:c1 * P, t,
                              sh * SH:(sh + 1) * SH].rearrange(
                                  "(c p) s -> p c s", p=P))

                def load_wt(eng, dst, src_d, t, c0, c1):
                    eng.dma_start(
                        dst[:, t, c0:c1, :],
                        src_d[c0 * P:c1 * P, t, :].rearrange(
                            "(c p) f -> p c f", p=P))

                # weights tiles ([partition, {hi,lo}, chunk, cols])
                wk2 = w_pool.tile([P, 2, KC, F], f8, tag="wk2")
                wq2 = w_pool.tile([P, 2, KC, F], f8, tag="wq2")
                wv2 = w_pool.tile([P, 2, KC, F], f8, tag="wv2")
                wo_sb = w_pool.tile([P, 4, D], b16, tag="wo")

                def act(name):
                    return acts_pool.tile([P, 2, KC, SH], f8, tag="acts",
                                          name=name)

                k0, k1 = act("k0"), act("k1")
                q0, q1 = act("q0"), act("q1")
                v0, v1 = act("v0"), act("v1")
                # DMA transfers are globally serial, so order the stream by
                # first use: hi pieces of wk+k0 first (the term-major K0
                # chain runs all hi*hi products before touching lo), then
                # the lo pieces, then k1, wq+q0, wv+v0, v1, q1, wo.
                for half in (0, 1):
                    c0, c1 = half * 4, half * 4 + 4
                    load_wt(nc.sync, wk2, wk_d, 0, c0, c1)
                    load_act(nc.scalar, k0, kin_d, 0, 0, c0, c1)
                for half in (0, 1):
                    c0, c1 = half * 4, half * 4 + 4
                    load_act(nc.scalar, k0, kin_d, 0, 1, c0, c1)
                    load_wt(nc.sync, wk2, wk_d, 1, c0, c1)
                nc.sync.dma_start(bk_sb[:], bk_d[:].rearrange("(m p) -> p m", p=P))
                nc.sync.dma_start(bq_sb[:], bq_d[:].rearrange("(m p) -> p m", p=P))
                for t in range(2):
                    load_act(nc.scalar, k1, kin_d, 1, t, 0, KC)
                nc.sync.dma_start(mk_sb[:], mk_d[:].rearrange("(c p) -> p c", p=P))
                # vones loads contiguously, then a DVE copy scatters it into
                # vsb's stride-65 mask column (a direct strided DMA costs
                # 3.6us of serial descriptor time)
                nc.sync.dma_start(vo_sb[:], vones_d[:])
                nc.vector.tensor_copy(vsb[:, :, :, HD], vo_sb[:])
                for t in range(2):
                    load_wt(nc.sync, wq2, wq_d, t, 0, KC)
                    load_act(nc.scalar, q0, qin_d, 0, t, 0, KC)
                for t in range(2):
                    load_wt(nc.sync, wv2, wv_d, t, 0, KC)
                    load_act(nc.scalar, v0, vin_d, 0, t, 0, KC)
                for t in range(2):
                    load_act(nc.scalar, v1, vin_d, 1, t, 0, KC)
                nc.sync.dma_start(bo_sb[:], bo_d[:].rearrange("(m p) -> p m", p=P))
                nc.sync.dma_start(id_sb[:], ident_d[:])
                for t in range(2):
                    load_act(nc.scalar, q1, qin_d, 1, t, 0, KC)
                nc.sync.dma_start(
                    wo_sb[:], wo_d[:].rearrange("(k p) f -> p k f", p=P))

                # ================= compute emitters =================
                TERMS = ((0, 0), (0, 1), (1, 0))  # (lhs hi/lo, rhs hi/lo)

                def emit_chain8(lhs, rhs, lcol, post):
                    # acc = sum over 4 chunk-pairs of the 3 fp8 cross terms
                    acc = flex.tile([P, SH], f32, tag="flex", name="acc")
                    i = 0
                    for p4 in range(4):
                        a, b = 2 * p4, 2 * p4 + 2
                        for li, ri in TERMS:
                            nc.tensor.matmul(
                                acc[:], lhs[:, li, a:b, lcol],
                                rhs[:, ri, a:b, :],
                                start=(i == 0), stop=(i == 11), perf_mode=DR)
                            i += 1
                    post(acc)

                def fix_kq(dst, sh, m, bias_sb):
                    def post(acc):
                        nc.vector.tensor_scalar(
                            dst[:, m, sh * SH:(sh + 1) * SH], acc[:],
                            bias_sb[:, m:m + 1], FIX, op0=ADD, op1=MUL)
                    return post

                def fix_v(sc):
                    def post(acc):
                        # mk pre-scaled by 2^-8 on the host
                        nc.vector.tensor_scalar(
                            vsb[:, sc, :, 0:HD],
                            acc[:].rearrange("p (h d) -> p h d", d=HD),
                            mk_sb[:, sc:sc + 1], None, op0=MUL)
                    return post

                def emit_kqchain(w2, x2, bias_sb, dst, sh, m):
                    emit_chain8(w2, x2, slice(m * P, (m + 1) * P),
                                fix_kq(dst, sh, m, bias_sb))

                def emit_vchain(x2, sh, s):
                    emit_chain8(x2, wv2, slice(s * P, (s + 1) * P),
                                fix_v(sh * 4 + s))

                def emit_k_termmajor(w2, src, bias_sb, dst, sh):
                    # all 4 m-chains term-major then pair-major: the hi*hi
                    # sweep only needs the hi pieces of the weights and
                    # activations (the first half of their DMA), so PE
                    # starts earlier
                    accs = [flex.tile([P, SH], f32, tag="flex", name="acc")
                            for _ in range(4)]
                    for ti, (li, ri) in enumerate(TERMS):
                        for p4 in range(4):
                            a, b = 2 * p4, 2 * p4 + 2
                            for m in range(4):
                                col = slice(m * P, (m + 1) * P)
                                nc.tensor.matmul(
                                    accs[m][:], w2[:, li, a:b, col],
                                    src[:, ri, a:b, :],
                                    start=(ti == 0 and p4 == 0),
                                    stop=(ti == 2 and p4 == 3), perf_mode=DR)
                    for m in range(4):
                        fix_kq(dst, sh, m, bias_sb)(accs[m])

                def emit_qk_pair(sh, h, ptt, cp):
                    # logits^T chunk-pair cp + exp into ptt
                    po = (h % 2) * HD
                    mq = h // 2
                    lg = pslg.tile([P, 2, SH], f32, tag="lg", name="lg")
                    for i in range(2):
                        c = 2 * cp + i
                        nc.tensor.matmul(
                            lg[:, i, :],
                            kt[po:po + HD, mq, c * P:(c + 1) * P],
                            qt[po:po + HD, mq, sh * SH:(sh + 1) * SH],
                            start=True, stop=True)
                    nc.scalar.activation(ptt[:, 2 * cp:2 * cp + 2, :], lg[:], Exp)

                def emit_av(sh, h, ptt):
                    # flipped AV: out[q, hd|denom]; denom comes from the mask
                    # column of V. Normalize per-partition into xn.
                    av = flex.tile([P, 4, HD + 1], f32, tag="flex", name="av")
                    for q4 in range(4):
                        for c in range(SC):
                            nc.tensor.matmul(
                                av[:, q4, :],
                                ptt[:, c, q4 * P:(q4 + 1) * P],
                                vsb[:, c, h, :],
                                start=(c == 0), stop=(c == SC - 1))
                    rcpt = rcp_pool.tile([P, 4], f32, tag="rcp", name="rcpt")
                    nc.vector.reciprocal(rcpt[:], av[:, :, HD])
                    xnt = xn_pool.tile([P, 4, HD], b16, tag="xn", name="xnt")
                    nc.vector.tensor_mul(
                        xnt[:], av[:, :, 0:HD],
                        rcpt[:].unsqueeze(2).broadcast_to([P, 4, HD]))
                    return xnt

                def emit_tr(sh, h, xnt):
                    # transpose xn [q, hd] -> xt [hd, q] (feature-major)
                    po = (h % 2) * HD
                    mq = h // 2
                    xtp = flex.tile([HD, SH], b16, tag="flex", name="xtp")
                    for q4 in range(4):
                        nc.tensor.matmul(
                            xtp[:, q4 * P:(q4 + 1) * P], xnt[:, q4, :], id_sb[:],
                            start=True, stop=True, is_transpose=True)
                    nc.vector.tensor_copy(
                        xt[po:po + HD, mq, sh * SH:(sh + 1) * SH], xtp[:])

                Copy = mybir.ActivationFunctionType.Copy

                def emit_o_stage1(sh, m):
                    # head-pairs 0,1 -> bf16 partial (bias folded in so the
                    # drain merge needs no scalar operand)
                    dst = po0 if sh == 0 else po1
                    acc = flex.tile([P, SH], f32, tag="flex", name="acc")
                    for hp in (0, 1):
                        nc.tensor.matmul(
                            acc[:], wo_sb[:, hp, m * P:(m + 1) * P],
                            xt[:, hp, sh * SH:(sh + 1) * SH],
                            start=(hp == 0), stop=(hp == 1))
                    nc.vector.tensor_scalar_add(dst[:, m, :], acc[:],
                                                bo_sb[:, m:m + 1])

                def emit_o_stage2(sh, m, on_act=False, split=False):
                    # head-pairs 2,3 + partial merge -> bf16 -> DMA
                    src = po0 if sh == 0 else po1
                    acc = flex.tile([P, SH], f32, tag="flex", name="acc")
                    nc.tensor.matmul(
                        acc[:], wo_sb[:, 2, m * P:(m + 1) * P],
                        xt[:, 2, sh * SH:(sh + 1) * SH],
                        start=True, stop=False)
                    nc.tensor.matmul(
                        acc[:], wo_sb[:, 3, m * P:(m + 1) * P],
                        xt[:, 3, sh * SH:(sh + 1) * SH],
                        start=False, stop=not on_act)
                    ob = ob_pool.tile([P, SH], b16, tag="ob", name="ob")
                    if on_act and split:
                        # very last chain: cast + DMA per half-column so the
                        # final DMA starts earlier and moves half the bytes
                        nc.tensor.matmul(acc[:], id_sb[:], src[:, m, :],
                                         start=False, stop=True)
                        for cl, ch in ((0, 256), (256, SH)):
                            nc.scalar.activation(ob[:, cl:ch],
                                                 acc[:, cl:ch], Copy)
                            nc.sync.dma_start(
                                out_d[m * P:(m + 1) * P,
                                      sh * SH + cl:sh * SH + ch],
                                ob[:, cl:ch])
                        return
                    if on_act:
                        # sh1 drain: add the partial on PE (identity matmul)
                        # and cast on the now-idle ACT engine, halving the
                        # serial DVE merge stream at the tail
                        nc.tensor.matmul(acc[:], id_sb[:], src[:, m, :],
                                         start=False, stop=True)
                        nc.scalar.activation(ob[:], acc[:], Copy)
                    else:
                        # partial already carries the bias
                        nc.vector.tensor_add(ob[:], acc[:], src[:, m, :])
                    # alternate output queues so neither serializes the tail
                    eng = nc.gpsimd if m % 2 == 0 else nc.sync
                    eng.dma_start(
                        out_d[m * P:(m + 1) * P, sh * SH:(sh + 1) * SH], ob[:])

                # ================= attention stream tables =================
                slots = [(s // NH, s % NH) for s in range(16)]
                av_sched = {6: [0, 1], 7: [2, 3], 8: [4, 5], 9: [6, 7],
                            10: [8], 11: [9, 10], 12: [11], 13: [12],
                            14: [13]}
                tr_sched = {7: [0, 1], 8: [2, 3], 9: [4, 5], 10: [6, 7],
                            11: [8], 12: [9, 10], 13: [11], 14: [12],
                            15: [13]}
                # all O work that does not need the last two heads is packed
                # before slot 15's QK pairs
                chain_sched = {
                    2: [("v", 0, 0), ("v", 0, 1)],
                    3: [("v", 0, 2), ("v", 0, 3)],
                    4: [("v", 1, 0), ("v", 1, 1)],
                    5: [("v", 1, 2), ("v", 1, 3)],
                    6: [("q1", 0)], 7: [("q1", 1)], 8: [("q1", 2)],
                    9: [("q1", 3), ("o0a", 0)],
                    10: [("o0a", 1), ("o0a", 2), ("o0a", 3)],
                    11: [("o0a", 4), ("o0a", 5), ("o0b", 0)],
                    12: [("o0a", 6), ("o0a", 7), ("o0b", 1)],
                    13: [("o0b", 2), ("o0b", 3), ("o1a", 0)],
                    14: [("o0b", 4), ("o0b", 5), ("o1a", 1), ("o1a", 2)],
                    15: [("o0b", 6)],
                }

                def emit_chain(spec):
                    kind = spec[0]
                    if kind == "v":
                        emit_vchain(v0 if spec[1] == 0 else v1,
                                    spec[1], spec[2])
                    elif kind == "q1":
                        emit_kqchain(wq2, q1, bq_sb, qt, 1, spec[1])
                    elif kind == "o0a":
                        emit_o_stage1(0, spec[1])
                    elif kind == "o0b":
                        emit_o_stage2(0, spec[1])
                    elif kind == "o1a":
                        emit_o_stage1(1, spec[1])

                def emit_av_tr_fine(sh, h, ptt, fillers=()):
                    # last-slot variant: per-q-tile AV -> rcp -> norm -> tr ->
                    # copy pipeline, so the tail latency is one q-tile.
                    po = (h % 2) * HD
                    mq = h // 2
                    av = flex.tile([P, 4, HD + 1], f32, tag="flex", name="av")
                    xtp = flex.tile([HD, SH], b16, tag="flex", name="xtp")
                    fillers = list(fillers)

                    def avmm(q4):
                        for c in range(SC):
                            nc.tensor.matmul(
                                av[:, q4, :],
                                ptt[:, c, q4 * P:(q4 + 1) * P],
                                vsb[:, c, h, :],
                                start=(c == 0), stop=(c == SC - 1))

                    def avq(q4):
                        rcpt = rcp_pool.tile([P, 1], f32, tag="rcp", name="rcpt")
                        nc.vector.reciprocal(rcpt[:], av[:, q4, HD:HD + 1])
                        xnt = xn_pool.tile([P, HD], b16, tag="xn", name="xnt")
                        nc.vector.tensor_scalar(
                            xnt[:], av[:, q4, 0:HD], rcpt[:], None, op0=MUL)
                        return xnt

                    def trq(q4, xnt):
                        nc.tensor.matmul(
                            xtp[:, q4 * P:(q4 + 1) * P], xnt[:], id_sb[:],
                            start=True, stop=True, is_transpose=True)
                        cs = slice(sh * SH + q4 * P, sh * SH + (q4 + 1) * P)
                        nc.vector.tensor_copy(xt[po:po + HD, mq, cs],
                                              xtp[:, q4 * P:(q4 + 1) * P])

                    def fill():
                        if fillers:
                            emit_chain(fillers.pop(0))

                    avmm(0)
                    xs = [avq(0)]
                    avmm(1)
                    fill()
                    xs.append(avq(1))
                    trq(0, xs[0])
                    avmm(2)
                    fill()
                    xs.append(avq(2))
                    trq(1, xs[1])
                    avmm(3)
                    fill()
                    xs.append(avq(3))
                    trq(2, xs[2])
                    fill()
                    trq(3, xs[3])

                pts = {}
                xns = {}

                def new_pt(s):
                    pts[s] = pt_pool.tile([P, SC, SH], b16, tag="pt",
                                          name="ptt")
                    return pts[s]

                # ============ phase A: K, V0, Q0 + first two slots ============
                # K0 first (term-major, DMA-paced), then K1, then V half 0
                # (its data lands during K1), then Q0 with slot 0/1's QK
                # pairs interleaved so the exp stream starts as soon as
                # qt m0 exists.
                emit_k_termmajor(wk2, k0, bk_sb, kt, 0)
                for m in range(4):
                    emit_kqchain(wk2, k1, bk_sb, kt, 1, m)
                new_pt(0)
                new_pt(1)
                emit_kqchain(wq2, q0, bq_sb, qt, 0, 0)
                emit_qk_pair(0, 0, pts[0], 0)
                emit_qk_pair(0, 0, pts[0], 1)
                emit_kqchain(wq2, q0, bq_sb, qt, 0, 1)
                emit_qk_pair(0, 0, pts[0], 2)
                emit_qk_pair(0, 0, pts[0], 3)
                emit_kqchain(wq2, q0, bq_sb, qt, 0, 2)
                emit_qk_pair(0, 1, pts[1], 0)
                emit_qk_pair(0, 1, pts[1], 1)
                emit_kqchain(wq2, q0, bq_sb, qt, 0, 3)
                emit_qk_pair(0, 1, pts[1], 2)
                emit_qk_pair(0, 1, pts[1], 3)

                for s, (sh, h) in list(enumerate(slots))[2:]:
                    ptt = new_pt(s)
                    avs = [iter(av_sched.get(s, []))]
                    trs = [iter(tr_sched.get(s, []))]
                    chains = list(chain_sched.get(s, []))

                    def fill(n_chains):
                        for t in avs[0]:
                            psh, ph = slots[t]
                            xns[t] = emit_av(psh, ph, pts[t])
                            del pts[t]
                            break
                        for t in trs[0]:
                            psh, ph = slots[t]
                            emit_tr(psh, ph, xns.pop(t))
                            break
                        for _ in range(n_chains):
                            if chains:
                                emit_chain(chains.pop(0))

                    emit_qk_pair(sh, h, ptt, 0)
                    emit_qk_pair(sh, h, ptt, 1)
                    fill(1)
                    emit_qk_pair(sh, h, ptt, 2)
                    fill(1)
                    emit_qk_pair(sh, h, ptt, 3)
                    # drain any remaining scheduled work for this block
                    for t in avs[0]:
                        psh, ph = slots[t]
                        xns[t] = emit_av(psh, ph, pts[t])
                        del pts[t]
                    for t in trs[0]:
                        psh, ph = slots[t]
                        emit_tr(psh, ph, xns.pop(t))
                    while chains:
                        emit_chain(chains.pop(0))
                    if s == 15:
                        emit_av_tr_fine(1, 6, pts.pop(14),
                                        fillers=[("o1a", 3), ("o1a", 4),
                                                 ("o1a", 5)])
                    if debug_taps and s == 2:
                        nc.sync.dma_start(dbg["pt0"][:], pts[2][:])

                # ================= drain =================
                emit_av_tr_fine(1, 7, pts.pop(15),
                                fillers=[("o1a", 6), ("o1a", 7),
                                         ("o0b", 7)])
                for m in range(4):
                    emit_o_stage2(1, m)
                for m in range(4, KC):
                    emit_o_stage2(1, m, on_act=True, split=(m == KC - 1))
                if debug_taps:
                    nc.sync.dma_start(dbg["kt"][:], kt[:])
                    nc.sync.dma_start(dbg["qt"][:], qt[:])
                    nc.sync.dma_start(dbg["vsb"][:], vsb[:])
                    nc.sync.dma_start(dbg["xt"][:], xt[:])

    nc.compile()
    return nc


_program = None
_last_in_maps = None


def _get_program():
    global _program
    if _program is None:
        _program = build_program()
    return _program


def _split_e4m3(x):
    """Interleaved hi/lo e4m3 split along a new axis 1: [d, 2, n]."""
    e4 = ml_dtypes.float8_e4m3
    x = np.ascontiguousarray(x)
    hi = x.astype(e4)
    lo = (x - hi.astype(np.float32)).astype(e4)
    return np.ascontiguousarray(np.stack([hi, lo], axis=1))


def kernel(inputs_q, inputs_kv, pos_emb_q, pos_emb_k, pos_emb_v,
           key_padding_mask, wq, bq, wk, bk, wv, bv, wo, bo):
    nc = _get_program()
    bf16 = ml_dtypes.bfloat16

    wqf = np.asarray(wq, np.float32).reshape(D, H * HD)
    wkf = np.asarray(wk, np.float32).reshape(D, H * HD)
    wvf = np.asarray(wv, np.float32).reshape(D, H * HD)
    wof = np.asarray(wo, np.float32).reshape(H * HD, D)
    bqf = np.asarray(bq, np.float32).reshape(H * HD)
    bkf = np.asarray(bk, np.float32).reshape(H * HD)
    bvf = np.asarray(bv, np.float32).reshape(H * HD)
    bof = np.asarray(bo, np.float32).reshape(D)
    # bv is structurally zero in this problem; it has no cheap slot in the
    # transposed dataflow, so refuse loudly rather than silently drop it.
    assert np.all(bvf == 0.0), "nonzero bv is not supported"

    scale = np.float32(1.0 / np.sqrt(HD))
    iq = np.asarray(inputs_q, np.float32)
    ikv = np.asarray(inputs_kv, np.float32)
    # positional embeddings folded on the host (f32, same math as reference)
    q_in = iq + np.asarray(pos_emb_q, np.float32)
    k_in = ikv + np.asarray(pos_emb_k, np.float32)
    v_in = ikv + np.asarray(pos_emb_v, np.float32)
    mask = np.asarray(key_padding_mask, np.float32)
    ident = np.eye(P, dtype=bf16)

    in_maps = []
    for b in range(B):
        q8 = _split_e4m3(q_in[b].T)
        k8 = _split_e4m3(k_in[b].T)
        v8 = _split_e4m3(v_in[b].T)
        mk = np.ascontiguousarray(mask[b])
        vones = np.ascontiguousarray(
            np.broadcast_to(mk.reshape(SC, P).T[:, :, None], (P, SC, NH))
        ).astype(bf16)
        for hg in range(2):
            sl = slice(hg * F, (hg + 1) * F)
            in_maps.append({
                "qin8": q8, "kin8": k8, "vin8": v8,
                "wq8": _split_e4m3(wqf[:, sl] * (256.0 * scale)),
                "wk8": _split_e4m3(wkf[:, sl] * 256.0),
                "wv8": _split_e4m3(wvf[:, sl] * 256.0),
                "wo": np.ascontiguousarray(wof[sl, :]).astype(bf16),
                "bq": np.ascontiguousarray(bqf[sl]) * (256.0 * scale),
                "bk": np.ascontiguousarray(bkf[sl]) * 256.0,
                "bo": bof if hg == 0 else np.zeros_like(bof),
                "mk": mk * np.float32(FIX),
                "vones": vones,
                "ident": ident,
            })

    global _last_in_maps
    _last_in_maps = in_maps
    res = run_bass_kernel_spmd(nc, in_maps, list(range(2 * B)))
    outs = [np.asarray(res.results[i]["out_t"], np.float32)
            for i in range(2 * B)]
    out = np.stack([(outs[2 * b] + outs[2 * b + 1]).T for b in range(B)])
    return np.ascontiguousarray(out, dtype=np.float32)
